# revision 34
# baseline (speedup 1.0000x reference)
"""Causal self-attention (RoPE) Trainium2 kernel, 8-way tensor-parallel.

Sharding (Megatron-style, zero-cost input distribution since every core
receives the full inputs): core c owns global heads {2c, 2c+1}.

Per core:
  1. qkv^T = W_slice^T @ x^T   (fp16 matmuls, fp32 psum), single pass over x^T;
     per token-chunk epilogue: bias add, RoPE on q/k (pair-swap DMA + 3 DVE
     ops), v PE-transposed to [token, d] layout
  2. Per (head, batch): causal flash-style attention
       scores psum [q,k] -> +mask -> exp (ACT, fused row-sum accum) ->
       normalize by 1/l (DVE) -> PE-transpose P-hat -> PV matmuls -> y^T [d,q]
  3. AllGather of y^T slices across the 8 cores, one per (head, batch),
     so projection work can start while later attention blocks still run
  4. Output projection vs the core's 256-column slice of W_proj in two
     accumulation halves (even heads -> fp32 partial in SBUF as soon as the
     even AGs land, odd heads + partial + bias -> fp32 out [tokens, 256])

Host side shards weights, builds RoPE/mask tables, and concatenates the
8 column slices into the final [B, T, C] output.
"""

import functools
import numpy as np

import concourse.bass as bass
import concourse.mybir as mybir
import concourse.tile as tile
from concourse import bacc
from concourse.bass_utils import run_bass_kernel_spmd
from concourse.masks import make_identity
from concourse.tile import add_dep_helper

F32 = mybir.dt.float32
F16 = mybir.dt.float16

N_CORES = 8
C = 2048           # model dim
H = 16             # total heads
HD = 128           # head dim
HL = 2             # heads per core
OC = C // N_CORES  # output cols per core (256)
SCALE = 1.0 / float(np.sqrt(HD))
MASK_VAL = -900.0  # additive pre-scale mask; exp arg ~ -80 -> underflows to 0


def build(B=2, T=2048, collective=True, n_cores=N_CORES):
    """Build the SPMD Bass program (identical on every core)."""
    BT = B * T
    NSTR = 3 * HL                  # qkv strips of 128 cols
    NCT = C // 128                 # contraction tiles
    NTCH = BT // 512               # token chunks for qkv
    NQC = T // 512                 # q chunks per (b, h)
    NTT = T // 128                 # token tiles per batch

    nc = bacc.Bacc(None, target_bir_lowering=False)
    xT = nc.dram_tensor("xT", [C, BT], F16, kind="ExternalInput")
    wqkv = nc.dram_tensor("wqkv", [C, NSTR * 128], F16, kind="ExternalInput")
    bqkv = nc.dram_tensor("bqkv", [NSTR * 128, 1], F32, kind="ExternalInput")
    ctil = nc.dram_tensor("ctil", [128, T], F16, kind="ExternalInput")
    stil = nc.dram_tensor("stil", [128, T], F16, kind="ExternalInput")
    wp = nc.dram_tensor("wp", [C, OC], F16, kind="ExternalInput")
    bpb = nc.dram_tensor("bpb", [128, OC], F32, kind="ExternalInput")
    cmask = nc.dram_tensor("cmask", [128, 128], F32, kind="ExternalInput")
    out = nc.dram_tensor("out", [BT, OC], F32, kind="ExternalOutput")

    with tile.TileContext(nc) as tc:
        with (
            tc.tile_pool(name="big", bufs=1) as big,
            tc.tile_pool(name="dram", bufs=1, space="DRAM") as dram,
        ):
            # ---- persistent SBUF tensors ----
            qr = big.tile([128, HL * BT], F16, tag="qr")
            kr = big.tile([128, HL * BT], F16, tag="kr")
            v_sb = big.tile([128, HL * BT], F16, tag="v_sb")
            ct_sb = big.tile([128, T], F16, tag="ct")
            st_sb = big.tile([128, T], F16, tag="st")
            ident = big.tile([128, 128], F16, tag="ident")
            cm_sb = big.tile([128, 128], F32, tag="cm")
            bq_sb = big.tile([128, NSTR], F32, tag="bq")
            bp_sb = big.tile([128, OC], F32, tag="bp")

            # DRAM bounce buffers: one AllGather per (local head j, batch b)
            agin = {}
            agout = {}
            for j in range(HL):
                for b in range(B):
                    agin[(j, b)] = dram.tile([128, T], F16, name=f"agin{j}_{b}")
                    agout[(j, b)] = dram.tile([n_cores * 128, T], F16,
                                              name=f"agout{j}_{b}")

            # ================= Phase A: QKV + RoPE + v-transpose =================
            with (
                tc.tile_pool(name="wq", bufs=1) as wq_pool,
                tc.tile_pool(name="xt", bufs=2) as xt_pool,
                tc.tile_pool(name="rope", bufs=2) as rope_pool,
                tc.tile_pool(name="stage", bufs=4) as stage_pool,
                tc.tile_pool(name="qkv_ps", bufs=2, space="PSUM") as qkv_ps,
                tc.tile_pool(name="vt_ps", bufs=2, space="PSUM") as vt_ps,
            ):
                # interleave first-chunk xT loads with the weight loads so the
                # first matmul group isn't queued behind 3 MB of weights
                w_sb = []
                xts_first = []
                for ctn in range(NCT):
                    xt_t = xt_pool.tile([128, 512], F16, tag=f"xt{ctn}",
                                        name=f"xt{ctn}")
                    nc.sync.dma_start(xt_t[:], xT[ctn * 128:(ctn + 1) * 128, 0:512])
                    xts_first.append(xt_t)
                    wt = wq_pool.tile([128, NSTR * 128], F16, tag=f"w{ctn}",
                                      name=f"w{ctn}")
                    nc.sync.dma_start(wt[:], wqkv[ctn * 128:(ctn + 1) * 128, :])
                    w_sb.append(wt)
                for s in range(NSTR):
                    nc.sync.dma_start(bq_sb[:, s:s + 1], bqkv[s * 128:(s + 1) * 128, :])
                make_identity(nc, ident[:])

                for tch in range(NTCH):
                    if tch == min(1, NTCH - 1):
                        # constants land after the first xT burst is in flight
                        nc.sync.dma_start(ct_sb[:], ctil[:, :])
                        nc.sync.dma_start(st_sb[:], stil[:, :])
                        nc.sync.dma_start(cm_sb[:], cmask[:, :])
                        nc.sync.dma_start(bp_sb[:], bpb[:, :])
                    tw = (tch * 512) % T        # token offset within batch
                    tok = slice(tw, tw + 512)
                    if tch == 0:
                        xts = xts_first
                    else:
                        xts = []
                        for ctn in range(NCT):
                            xt_t = xt_pool.tile([128, 512], F16, tag=f"xt{ctn}",
                                                name=f"xt{ctn}")
                            nc.sync.dma_start(
                                xt_t[:],
                                xT[ctn * 128:(ctn + 1) * 128,
                                   tch * 512:(tch + 1) * 512])
                            xts.append(xt_t)
                    stg = {}
                    for s in range(NSTR):
                        ps = qkv_ps.tile([128, 512], F32, name="qkvps")
                        for ctn in range(NCT):
                            nc.tensor.matmul(
                                ps[:], w_sb[ctn][:, s * 128:(s + 1) * 128], xts[ctn][:],
                                start=(ctn == 0), stop=(ctn == NCT - 1))
                        kindtag = ("qs0", "qs1", "ks0", "ks1", "vs0", "vs1")[s]
                        st_t = stage_pool.tile([128, 512], F16, tag=kindtag,
                                               name=kindtag)
                        nc.scalar.activation(
                            st_t[:], ps[:], mybir.ActivationFunctionType.Identity,
                            bias=bq_sb[:, s:s + 1], scale=1.0)
                        stg[s] = st_t
                    for j in range(HL):
                        # RoPE on q and k for this (j, tch)
                        for st_t, dst, swtag in ((stg[j], qr, f"swq{j}"),
                                                 (stg[2 + j], kr, f"swk{j}")):
                            dstsl = dst[:, j * BT + tch * 512: j * BT + (tch + 1) * 512]
                            sw = rope_pool.tile([128, 512], F16, tag=swtag, name=swtag)
                            nc.sync.dma_start(sw[0:127:2, :], st_t[1:128:2, :])
                            nc.sync.dma_start(sw[1:128:2, :], st_t[0:127:2, :])
                            tmp = rope_pool.tile([128, 512], F16, tag=swtag + "t",
                                                 name=swtag + "t")
                            nc.vector.tensor_mul(dstsl, st_t[:], ct_sb[:, tok])
                            nc.vector.tensor_mul(tmp[:], sw[:], st_sb[:, tok])
                            nc.vector.tensor_add(dstsl, dstsl, tmp[:])
                        # v transpose for this (j, tch)
                        for blk in range(4):
                            tt = tch * 4 + blk
                            ps = vt_ps.tile([128, 128], F16, tag="vtp", name="vtp")
                            nc.tensor.transpose(
                                ps[:], stg[4 + j][:, blk * 128:(blk + 1) * 128],
                                ident[:])
                            nc.scalar.copy(
                                v_sb[:, j * BT + tt * 128: j * BT + (tt + 1) * 128],
                                ps[:])

            # ============ Phase B: attention + AG + interleaved projection ============
            with (
                tc.tile_pool(name="p", bufs=4) as p_pool,
                tc.tile_pool(name="pt", bufs=3) as pt_pool,
                tc.tile_pool(name="stat", bufs=8) as stat_pool,
                tc.tile_pool(name="yts", bufs=4) as yts_pool,
                tc.tile_pool(name="wpp", bufs=1) as wp_pool,
                tc.tile_pool(name="ygs", bufs=2) as ygs_pool,
                tc.tile_pool(name="part", bufs=1) as part_pool,
                tc.tile_pool(name="ot", bufs=4) as ot_pool,
                tc.tile_pool(name="sc_ps", bufs=2, space="PSUM") as sc_ps,
                tc.tile_pool(name="tp_ps", bufs=2, space="PSUM") as tp_ps,
                tc.tile_pool(name="y_ps", bufs=2, space="PSUM") as y_ps,
                tc.tile_pool(name="o_ps", bufs=1, space="PSUM") as o_ps,
            ):
                wp_sb = []
                for g in range(H):
                    wt = wp_pool.tile([128, OC], F16, tag=f"wp{g}", name=f"wp{g}")
                    nc.sync.dma_start(wt[:], wp[g * 128:(g + 1) * 128, :])
                    wp_sb.append(wt)
                partial = {}
                cc_insts = {}
                agin_dmas = {}

                def attention_block(j, b):
                    base = j * BT + b * T
                    for qc in range(NQC):
                        nkt = 4 * qc + 4
                        ptall = pt_pool.tile([128, 8192], F16, tag="ptall",
                                             name="ptall")
                        for qtw in range(4):
                            qt = qc * 4 + qtw
                            kext = (qt + 1) * 128
                            qtile = qr[:, base + qt * 128: base + (qt + 1) * 128]
                            ptile = p_pool.tile([128, T], F16, tag="P", name="P")
                            lparts = []
                            off = 0
                            while off < kext:
                                n = min(512, kext - off)
                                ps = sc_ps.tile([128, 512], F32, name="scps")
                                nc.tensor.matmul(
                                    ps[:, :n], qtile, kr[:, base + off: base + off + n],
                                    start=True, stop=True)
                                if off + n == kext:
                                    nc.vector.tensor_add(
                                        ps[:, n - 128:n], ps[:, n - 128:n], cm_sb[:])
                                lp = stat_pool.tile([128, 1], F32, tag="lp", name="lp")
                                nc.scalar.activation(
                                    ptile[:, off:off + n], ps[:, :n],
                                    mybir.ActivationFunctionType.Exp,
                                    scale=SCALE, accum_out=lp[:])
                                lparts.append(lp)
                                off += n
                            lsum = stat_pool.tile([128, 1], F32, tag="ls", name="ls")
                            if len(lparts) == 1:
                                lsum = lparts[0]
                            else:
                                nc.vector.tensor_add(lsum[:], lparts[0][:], lparts[1][:])
                                for lp in lparts[2:]:
                                    nc.vector.tensor_add(lsum[:], lsum[:], lp[:])
                            rec = stat_pool.tile([128, 1], F32, tag="rec", name="rec")
                            nc.vector.reciprocal(rec[:], lsum[:])
                            nc.vector.tensor_scalar_mul(
                                ptile[:, :kext], ptile[:, :kext], rec[:])
                            kt = 0
                            while kt <= qt:
                                if kt + 1 <= qt:
                                    tp = tp_ps.tile([128, 256], F16, tag="tp", name="tp")
                                    nc.tensor.transpose(
                                        tp[:, 0:128],
                                        ptile[:, kt * 128:(kt + 1) * 128], ident[:])
                                    nc.tensor.transpose(
                                        tp[:, 128:256],
                                        ptile[:, (kt + 1) * 128:(kt + 2) * 128],
                                        ident[:])
                                    dst = ptall[:].rearrange(
                                        "p (a b) -> p a b", b=512)[
                                        :, kt:kt + 2, qtw * 128:(qtw + 1) * 128]
                                    nc.vector.tensor_copy(
                                        dst,
                                        tp[:].rearrange("p (a b) -> p a b", a=2))
                                    kt += 2
                                else:
                                    tp = tp_ps.tile([128, 256], F16, tag="tp", name="tp")
                                    nc.tensor.transpose(
                                        tp[:, 0:128],
                                        ptile[:, kt * 128:(kt + 1) * 128], ident[:])
                                    nc.vector.tensor_copy(
                                        ptall[:, kt * 512 + qtw * 128:
                                              kt * 512 + (qtw + 1) * 128],
                                        tp[:, 0:128])
                                    kt += 1
                        psy = y_ps.tile([128, 512], F32, name="psy")
                        for kt in range(nkt):
                            qstart = max(0, (kt - 4 * qc)) * 128
                            nc.tensor.matmul(
                                psy[:, qstart:512],
                                v_sb[:, base + kt * 128: base + (kt + 1) * 128],
                                ptall[:, kt * 512 + qstart: kt * 512 + 512],
                                start=(kt == 0), stop=(kt == nkt - 1))
                        yt = yts_pool.tile([128, 512], F16, tag="yt", name="yt")
                        nc.vector.tensor_copy(yt[:], psy[:])
                        d = nc.sync.dma_start(
                            agin[(j, b)][:, qc * 512:(qc + 1) * 512], yt[:])
                        agin_dmas.setdefault((j, b), []).append(d)
                    if collective:
                        cc = nc.gpsimd.collective_compute(
                            "AllGather",
                            mybir.AluOpType.bypass,
                            replica_groups=[list(range(n_cores))],
                            ins=[agin[(j, b)].opt()],
                            outs=[agout[(j, b)].opt()],
                        )
                    else:
                        cc = nc.sync.dma_start(agout[(j, b)][0:128, :],
                                               agin[(j, b)][:, :])
                    cci = cc.ins if hasattr(cc, "ins") else cc
                    for d in agin_dmas[(j, b)]:
                        di = d.ins if hasattr(d, "ins") else d
                        add_dep_helper(cci, di,
                                       reason="collective reads agin after y writes")
                    cc_insts[(j, b)] = cci

                def proj_half(b, par, first):
                    """Accumulate heads of parity `par` for batch b.

                    first=True: psum + bias -> fp32 partial tiles in SBUF.
                    first=False: psum + partial -> output DMA.
                    """
                    gs = list(range(par, H, 2))
                    for tg0 in range(0, NTT, 4):
                        ng = min(4, NTT - tg0)
                        ygq = {}
                        for g in gs:
                            row = (g // 2) * 128
                            yg = ygs_pool.tile([128, ng * 128], F16, tag=f"yg{g}",
                                               name=f"yg{g}")
                            d = nc.sync.dma_start(
                                yg[:],
                                agout[(g % 2, b)][row:row + 128,
                                                  tg0 * 128:(tg0 + ng) * 128])
                            di = d.ins if hasattr(d, "ins") else d
                            add_dep_helper(di, cc_insts[(g % 2, b)],
                                           reason="proj reads agout after collective")
                            ygq[g] = yg
                        for p0 in range(0, ng, 2):
                            npair = min(2, ng - p0)
                            pss = [o_ps.tile([128, OC], F32, tag=f"op{i}",
                                             name=f"op{i}")
                                   for i in range(npair)]
                            for gi, g in enumerate(gs):
                                for i in range(npair):
                                    nc.tensor.matmul(
                                        pss[i][:],
                                        ygq[g][:, (p0 + i) * 128:(p0 + i + 1) * 128],
                                        wp_sb[g][:],
                                        start=(gi == 0), stop=(gi == len(gs) - 1))
                            for i in range(npair):
                                tt = tg0 + p0 + i
                                if first:
                                    pt_t = part_pool.tile([128, OC], F32,
                                                          tag=f"part{b}_{tt}",
                                                          name=f"part{b}_{tt}")
                                    nc.vector.tensor_add(pt_t[:], pss[i][:], bp_sb[:])
                                    partial[(b, tt)] = pt_t
                                else:
                                    ot = ot_pool.tile([128, OC], F32, tag="ot",
                                                      name="ot")
                                    nc.vector.tensor_add(
                                        ot[:], pss[i][:], partial[(b, tt)][:])
                                    r0 = b * T + tt * 128
                                    nc.sync.dma_start(out[r0:r0 + 128, :], ot[:])

                for b in range(B):
                    attention_block(0, b)
                for b in range(B):
                    proj_half(b, 0, first=True)
                for b in range(B):
                    attention_block(1, b)
                    proj_half(b, 1, first=False)
    nc.compile()
    return nc


def _prep_inputs(x, W_attn, b_attn, W_proj, b_proj, cos, sin, core, B, T):
    """Host-side shard prep for one core."""
    BT = B * T
    xT = np.ascontiguousarray(x.reshape(BT, C).T).astype(np.float16)

    cols = []
    bvals = []
    for part in range(3):  # q, k, v
        for j in range(HL):
            h = 2 * core + j
            sl = slice(part * C + h * HD, part * C + (h + 1) * HD)
            cols.append(W_attn[:, sl])
            bvals.append(b_attn[sl])
    wqkv = np.concatenate(cols, axis=1).astype(np.float16)
    bqkv = np.concatenate(bvals).astype(np.float32).reshape(-1, 1)

    # RoPE tables: ctil[p, t] = cos[t, p//2]; stil[2i] = -sin, stil[2i+1] = +sin
    cosr = np.repeat(cos.T, 2, axis=0)  # [128, T]
    sinr = np.repeat(sin.T, 2, axis=0)
    sgn = np.where((np.arange(128) % 2) == 0, -1.0, 1.0)[:, None]
    ctil = cosr.astype(np.float16)
    stil = (sinr * sgn).astype(np.float16)

    wp_c = W_proj[:, core * OC:(core + 1) * OC].astype(np.float16)
    bpb = np.tile(b_proj[core * OC:(core + 1) * OC].astype(np.float32), (128, 1))
    ii, jj = np.mgrid[0:128, 0:128]
    cmask = np.where(jj <= ii, 0.0, MASK_VAL).astype(np.float32)
    return {
        "xT": xT, "wqkv": wqkv, "bqkv": bqkv, "ctil": ctil, "stil": stil,
        "wp": wp_c, "bpb": bpb, "cmask": cmask,
    }


@functools.lru_cache(maxsize=2)
def _built(B, T):
    return build(B=B, T=T)


_warmed = set()


def kernel(x, W_attn, b_attn, W_proj, b_proj, cos, sin):
    x = np.asarray(x, dtype=np.float32)
    W_attn = np.asarray(W_attn, dtype=np.float32)
    b_attn = np.asarray(b_attn, dtype=np.float32)
    W_proj = np.asarray(W_proj, dtype=np.float32)
    b_proj = np.asarray(b_proj, dtype=np.float32)
    cos = np.asarray(cos, dtype=np.float32)
    sin = np.asarray(sin, dtype=np.float32)

    B, T, Cv = x.shape
    assert Cv == C
    nc = _built(B, T)
    in_maps = [_prep_inputs(x, W_attn, b_attn, W_proj, b_proj, cos, sin, c, B, T)
               for c in range(N_CORES)]
    if (B, T) not in _warmed:
        # The very first execution of a freshly loaded NEFF has been observed
        # to deliver stale/uninitialized collective buffers; run once and
        # discard, then run for real.
        run_bass_kernel_spmd(nc, in_maps, core_ids=list(range(N_CORES)))
        _warmed.add((B, T))
    res = run_bass_kernel_spmd(nc, in_maps, core_ids=list(range(N_CORES)))
    outs = [res.results[c]["out"] for c in range(N_CORES)]
    full = np.concatenate(outs, axis=1)  # [BT, C]
    return full.reshape(B, T, C).astype(np.float32)
